# revision 42
# baseline (speedup 1.0000x reference)
"""Trainium2 Bass kernel for nn_AttentionCrossLayer.

Math: in the reference, softmax over a length-1 axis is exactly 1.0, so
attn == v and q/k/wq/wk are dead code. With x0 the (never-mutated) input,
each layer's gate xw_i = out_i @ cw_i is a fixed linear function of x0:
    xw_i = x0 @ u_i + c_i,   u_i = Wv_i @ (Wo_i @ cw_i),
                             c_i = (bv_i @ Wo_i + bo_i) @ cw_i
and the layer recurrence x += x0 * xw_i + cb_i telescopes to
    out[b, d] = x0[b, d] * (x0[b, :] @ usum + cprime) + cbsum[d]
with usum = sum_i u_i  [D], cprime = 1 + sum_i c_i, cbsum = sum_i cb_i [D].

The tiny weight contractions happen host-side in float64. The rel-err
gate is 2e-2, so x is staged to the device in bf16 and the output is
stored in bf16 (upcast to f32 on the host): this halves HBM traffic to
16.8MB/core. Quantization error ~0.2% RMS; measured rel err 2.5e-3.

Layout: 2 consecutive x rows per SBUF partition (tile = [128, 2048]
covering 256 rows) so every DMA descriptor is a contiguous 4KB DRAM
line (2KB lines pay ~2x per-descriptor overhead on the 16 shared DMA
engines). All 16 tiles stay SBUF-resident. Slot layout per partition
(bf16 elements, stride 2176 = 128B aligned):
  [pad | c@62 c@63 | row0 (1024) | row1 (1024) | c@2112 c@2113 | pad]
with 1.0 constants in the four marked cells. The two per-tile reduce
windows are 1026 wide ([62..1088) and [1088..2114)): each covers its
row plus TWO adjacent constant cells, and the matching u operand is
the broadcast row [cA, cB, usum, cA, cB] read at offset 0 resp. 2,
where cA + cB is a two-term bf16 split of cprime. The reduce thus
emits the finished gate t = x.usum + cprime with no extra add op.

Engine split (measured: the fused multiply+reduce is pinned at
1 elem/lane/cycle, ~1.14us per window, even with all operands 2-byte
and 4B-aligned — the accumulator path never enters the DVE's 2x mode):
the DVE runs all 32 reduce windows plus the last tile's gate-multiply
(bf16 tensor_scalar, ~0.4us); the Scalar engine (activation with a
per-partition f32 scale AP, 1 elem/cycle, ~1.16us per chunk) covers
tiles 0..14 and issues the trailing two stores on the by-then-idle
HWDGE path; GpSimd (SWDGE) issues stores 0..13. Both compute engines
run ~96% busy; the DVE's 32 windows are the critical path.

Schedule notes baked in from trace analysis:
- u is staged host-replicated as [128, D+4] and loaded with a straight
  per-partition DMA issued FIRST: a [0,P]-stride broadcast AP makes
  all 128 descriptors read the same DRAM line and serializes ~3us on
  one DRAM page, parking the DVE (gated on u) until ~13.7us.
- A dummy activation warms the Scalar engine's table (ACT_TABLE_LOAD,
  1.3us) off the critical path before the first real gate-multiply.
- GpSimd MUST await its SWDGE store completions before block end: the
  end-of-block drain resets SWDGE semaphore tracking and doing so with
  stores in flight faults the device (NRT_EXEC_UNIT_UNRECOVERABLE).
  The HWDGE trailing stores are deliberately NOT awaited: they drain
  under the fixed ~7us end-of-block semaphore walk (safe, measured
  over repeated runs; only the SWDGE path faults).
- DVE/Scalar instructions do NOT interlock RAW across the pipe: every
  read of an accum output or memset constant goes through a semaphore.

Sharding: data-parallel over batch across 8 cores, weights replicated,
no cross-device comms.
"""

import numpy as np

L, B, D, H, K = 3, 32768, 1024, 8, 64
N_CORES = 8
B_LOC = B // N_CORES  # 4096 rows per core
P = 128
R = 2
N_TILES = B_LOC // (P * R)  # 16
FREE = R * D
XOFF = 64  # data offset inside a slot; cells 62,63 are chunk-0 constants
C1 = XOFF + FREE  # cells 2112,2113 are chunk-1 constants
DPP = 2176  # slot stride in elements; 4352B = 128B aligned
W2 = D + 2  # reduce window width (row + two constant cells)
TM = 15  # first tile whose pass-2 runs on the DVE

_cache = {}


def _build_program(zero_cb: bool):
    import concourse.bass as bass
    from concourse import mybir

    F32 = mybir.dt.float32
    BF16 = mybir.dt.bfloat16
    MUL = mybir.AluOpType.mult
    ADD = mybir.AluOpType.add

    nc = bass.Bass()
    x = nc.declare_dram_parameter("x", [N_TILES * P, FREE], BF16, isOutput=False)
    u = nc.declare_dram_parameter("u", [P, D + 4], BF16, isOutput=False)
    cb = nc.declare_dram_parameter("cb", [1, D], F32, isOutput=False)
    out = nc.declare_dram_parameter("out", [N_TILES * P, FREE], BF16, isOutput=True)

    u_bcast = u.ap()  # host-replicated: each partition streams its own line
    cb_bcast = bass.AP(tensor=cb.ap().tensor, offset=0, ap=[[0, P], [1, D]])

    LAST = N_TILES - 1

    with (
        nc.sbuf_tensor([P, D + 4], BF16) as ub,  # [cA, cB, usum, cA, cB]
        nc.sbuf_tensor([P, D], F32) as cbb,
        nc.sbuf_tensor([P, N_TILES, DPP], BF16) as xt,
        # throwaway STT main outs; one slot per (tile, chunk) so no WAW
        # ordering is needed (the 8-deep DVE pipe would otherwise race)
        nc.sbuf_tensor([P, N_TILES, R, W2], BF16) as oscr,
        nc.sbuf_tensor([P, N_TILES, R], F32) as tsc,  # finished gates
        nc.sbuf_tensor([P, 1], BF16) as warm,  # act-table warmup scratch
        nc.semaphore("us") as us,
        nc.semaphore("ld0b") as ld0b,  # tile 0 odd-row half landed
        nc.semaphore("cm") as cm,    # STT accum writebacks retired (DVE)
        nc.semaphore("cm2") as cm2,  # Scalar-owned tiles scaled
        nc.semaphore("cm3") as cm3,  # DVE-owned tiles scaled
        nc.semaphore("st") as st,    # SWDGE store DMAs retired
        nc.semaphore("st2") as st2,  # HWDGE trailing stores retired
        nc.Block() as block,
    ):
        lds = [nc.alloc_semaphore(f"ld{i}") for i in range(N_TILES)]

        @block.sync
        def _(sync):
            # tile 0 arrives as two half-loads (even rows then odd rows,
            # 2KB descriptors) so the first reduce window can start after
            # only half the tile has landed
            ev = bass.AP(tensor=x.ap().tensor, offset=0, ap=[[2 * D, P], [1, D]])
            od = bass.AP(tensor=x.ap().tensor, offset=D, ap=[[2 * D, P], [1, D]])
            sync.dma_start(out=xt[:, 0, XOFF : XOFF + D], in_=ev).then_inc(lds[0], 16)
            sync.dma_start(out=xt[:, 0, XOFF + D : C1], in_=od).then_inc(ld0b, 16)
            for i in range(1, N_TILES):
                sync.dma_start(
                    out=xt[:, i, XOFF:C1], in_=x[i * P : (i + 1) * P, :]
                ).then_inc(lds[i], 16)

        @block.vector
        def _(vector):
            # 1.0 constants adjacent to each reduce window; they ride the
            # cm chain (DVE has no same-engine RAW interlock)
            nc.vector.memset(xt[:, :, XOFF - 2 : XOFF], 1.0).then_inc(cm, 1)
            nc.vector.memset(xt[:, :, C1 : C1 + 2], 1.0).then_inc(cm, 1)
            vector.wait_ge(us, 16 if zero_cb else 32)
            vector.wait_ge(cm, 2)
            for i in range(N_TILES):
                vector.wait_ge(lds[i], 16)
                for r in range(R):
                    if i == 0 and r == 1:
                        vector.wait_ge(ld0b, 16)
                    # oscr = win * u'; tsc[i,r] = sum = x_r . usum + cprime
                    nc.vector.scalar_tensor_tensor(
                        out=oscr[:, i, r, :],
                        in0=xt[:, i, XOFF - 2 + r * W2 : XOFF - 2 + (r + 1) * W2],
                        scalar=1.0,
                        in1=ub[:, 2 * r : 2 * r + W2],
                        op0=MUL,
                        op1=MUL,
                        accum_out=tsc[:, i, r : r + 1],
                    ).then_inc(cm, 1)
                if not zero_cb:
                    vector.wait_ge(cm, 2 + R * (i + 1))
                    for r in range(R):
                        nc.vector.scalar_tensor_tensor(
                            out=xt[:, i, XOFF + r * D : XOFF + (r + 1) * D],
                            in0=xt[:, i, XOFF + r * D : XOFF + (r + 1) * D],
                            scalar=tsc[:, i, r : r + 1],
                            in1=cbb[:, :],
                            op0=MUL,
                            op1=ADD,
                        ).then_inc(cm2, 1)
                elif i == LAST:
                    # tile 15 pass 2 on the DVE (~0.4us per chunk); the
                    # self-wait makes the accum writebacks retire before
                    # the gates are read
                    vector.wait_ge(cm, 2 + R * (i + 1))
                    for r in range(R):
                        nc.vector.tensor_scalar_mul(
                            out=xt[:, i, XOFF + r * D : XOFF + (r + 1) * D],
                            in0=xt[:, i, XOFF + r * D : XOFF + (r + 1) * D],
                            scalar1=tsc[:, i, r : r + 1],
                        ).then_inc(cm3, 1)

        @block.scalar
        def _(scalar):
            if zero_cb:
                # load the activation table off the critical path; read a
                # cell the u-broadcast initialized, write dead scratch
                scalar.wait_ge(us, 16)
                nc.scalar.mul(out=warm[:, :], in_=ub[:, 0:1], mul=1.0)
                # pass 2 for tiles 0..14 plus tile 15's chunk 0
                # (per-partition f32 scale AP)
                for i in range(TM):
                    for r in range(R):
                        scalar.wait_ge(cm, 2 + R * i + r + 1)
                        nc.scalar.mul(
                            out=xt[:, i, XOFF + r * D : XOFF + (r + 1) * D],
                            in_=xt[:, i, XOFF + r * D : XOFF + (r + 1) * D],
                            mul=tsc[:, i, r : r + 1],
                        ).then_inc(cm2, 1)
                # trailing stores on the idle HWDGE path; self-wait on
                # cm2 (own muls retired), cm3 for the DVE-scaled tile
                scalar.wait_ge(cm2, R * (N_TILES - 1))
                scalar.dma_start(
                    out=out[(N_TILES - 2) * P : (N_TILES - 1) * P, :],
                    in_=xt[:, N_TILES - 2, XOFF:C1],
                ).then_inc(st2, 16)
                scalar.wait_ge(cm3, R)
                scalar.dma_start(
                    out=out[LAST * P :, :], in_=xt[:, LAST, XOFF:C1]
                ).then_inc(st2, 16)
                # NO wait on st2: HWDGE transfers drain under the
                # end-of-block barrier/walk (SWDGE still awaited below)

        @block.gpsimd
        def _(gpsimd):
            # u rides GpSimd's SWDGE path: this engine wakes ~1us earlier
            # than sync and its queue has no x-load descriptors ahead
            gpsimd.dma_start(out=ub[:, :], in_=u_bcast).then_inc(us, 16)
            if not zero_cb:
                gpsimd.dma_start(out=cbb[:, :], in_=cb_bcast).then_inc(us, 16)
            n_sw = N_TILES - 2 if zero_cb else N_TILES
            for i in range(n_sw):
                if zero_cb and i >= TM:
                    gpsimd.wait_ge(cm3, R * (i - TM + 1))
                else:
                    gpsimd.wait_ge(cm2, R * (i + 1))
                gpsimd.dma_start(
                    out=out[i * P : (i + 1) * P, :], in_=xt[:, i, XOFF:C1]
                ).then_inc(st, 16)
            # SWDGE transfers MUST be awaited before block end (the drain
            # resets SWDGE semaphore tracking; in-flight stores fault)
            gpsimd.wait_ge(st, 16 * n_sw)

    return nc


def _precompute(wv, bv, wo, bo, cw, cb):
    """Host-side f64 contraction of the small per-layer weights."""
    usum = np.zeros(D, np.float64)
    cprime = 1.0
    for i in range(L):
        Wv = wv[i].reshape(D, H * K).astype(np.float64)
        Wo = wo[i].reshape(H * K, D).astype(np.float64)
        cwi = cw[i].reshape(D).astype(np.float64)
        wocw = Wo @ cwi
        usum += Wv @ wocw
        cprime += float(bv[i].reshape(H * K).astype(np.float64) @ wocw)
        cprime += float(bo[i].astype(np.float64) @ cwi)
    cbsum = cb.astype(np.float64).sum(axis=0)
    return usum.astype(np.float32), float(cprime), cbsum.astype(np.float32)


def _ensure_trace_hook_importable():
    # bass_utils unconditionally imports antenv.axon_hooks when the
    # BASS_TRACE env var is set; some images lack that module. A None
    # hook makes bass_utils skip tracing gracefully.
    try:
        import antenv.axon_hooks  # noqa: F401
    except ImportError:
        import sys
        import types

        mod = types.ModuleType("antenv.axon_hooks")
        mod.get_axon_ntff_profile_hook = lambda: None
        mod.set_axon_ntff_profile_hook = lambda hook: None
        sys.modules["antenv.axon_hooks"] = mod


def kernel(x, wq, bq, wk, bk, wv, bv, wo, bo, cw, cb):
    import ml_dtypes

    from concourse.bass_utils import run_bass_kernel_spmd

    _ensure_trace_hook_importable()

    bf16 = np.dtype(ml_dtypes.bfloat16)
    x = np.ascontiguousarray(np.asarray(x, dtype=np.float32)).astype(bf16)
    usum, cprime, cbsum = _precompute(
        np.asarray(wv), np.asarray(bv), np.asarray(wo), np.asarray(bo),
        np.asarray(cw), np.asarray(cb),
    )
    zero_cb = not np.any(cbsum)

    if zero_cb not in _cache:
        _cache[zero_cb] = _build_program(zero_cb)
    nc = _cache[zero_cb]

    # two-term bf16 split: cA + cB == cprime to ~1e-5
    cA = np.float32(cprime).astype(bf16)
    cB = np.float32(cprime - float(cA)).astype(bf16)
    u2 = np.concatenate(
        [[cA, cB], usum.astype(bf16), [cA, cB]]
    ).astype(bf16).reshape(1, D + 4)
    u2 = np.ascontiguousarray(np.broadcast_to(u2, (P, D + 4)))
    cb2 = cbsum.reshape(1, D)
    in_maps = [
        {
            "x": x[c * B_LOC : (c + 1) * B_LOC].reshape(N_TILES * P, FREE),
            "u": u2,
            "cb": cb2,
        }
        for c in range(N_CORES)
    ]
    res = run_bass_kernel_spmd(nc, in_maps, list(range(N_CORES)))
    out16 = np.concatenate(
        [res.results[c]["out"].reshape(B_LOC, D) for c in range(N_CORES)], axis=0
    )
    return out16.astype(np.float32)
